# revision 7
# baseline (speedup 1.0000x reference)
"""Trainium2 Bass kernel for the water-network leak MSE model.

Math (reference):
    net(s)   = base[idx_s] + MLP(idx_s)                    (idx_s in [0,1024))
    y        = net*onehot(idx) @ M^T + demand              demand[:, 2j] = D[:, j]
    q        = y @ inv
    hL       = sign(q) * K * |q|^1.852,  K = 10.667 C^-1.852 d^-4.871 L
    H        = (supply - hL) @ inv^T
    d_leak   = Cd*a*sqrt(2g) * (onehot @ M^T) * sqrt(relu(H))
    out      = mean((q @ A0^T - demand - d_leak)^2)

Device strategy (8 cores, data-parallel over samples, 2048 samples/core):
  All sample-independent weight transforms are folded on the host:
    PM  = inv^T M  (so q = net * PM[:, idx] + D @ inv_even),
    AM  = A0' PM   (so q @ A0'^T = net * AM[:, idx] + D @ (A0' inv_even^T)^T),
  with the per-pipe net table pre-multiplied into PM/AM columns. Node rows are
  permuted even-first so the demand subtraction is a contiguous slice.
  On device, per 512-sample chunk (features on partitions, samples on free):
    - one transposed dma_gather pulls M^T/PM^T/AM^T columns for the chunk's
      leak ids directly into [feature, sample] layout (bf16),
    - PE: D-part matmuls (K=256) for q and the residual, identity-matmul
      injects of the gathered parts into PSUM, and the full H matmul (K=1024),
    - ACT: |q| -> ln -> exp(0.852*ln + lnK) power chain, relu(hsup - H),
      sqrt via exp(0.5*ln), and Square+accumulate for the MSE partials,
    - DVE: hL = q * e, d_leak, residual assembly.
  Each core returns [128, 16] partial sums of squares; host reduces.
"""

import math

import numpy as np
import ml_dtypes

P = 128
N_CORES = 8
S_TOTAL = 16384
SC = S_TOTAL // N_CORES  # samples per core
CH = 512                 # samples per chunk
NCH = SC // CH           # chunks per core
N_NODES = 512
N_PIPES = 1024
N_DEM = 256
G_ACC = 9.80665

BF16 = ml_dtypes.bfloat16

_MODULE_CACHE: dict = {}


def _build_module():
    import concourse.bacc as bacc
    import concourse.mybir as mybir
    import concourse.tile as tile

    f32 = mybir.dt.float32
    bf16 = mybir.dt.bfloat16
    i16 = mybir.dt.int16
    AF = mybir.ActivationFunctionType
    OP = mybir.AluOpType

    nc = bacc.Bacc(trn_type="TRN2", target_bir_lowering=False, debug=False)

    maux = nc.dram_tensor("maux", [N_PIPES, 2048], bf16, kind="ExternalInput").ap()
    invev_d = nc.dram_tensor("invev", [P, 16 * P], bf16, kind="ExternalInput").ap()
    invpt_d = nc.dram_tensor("invpt", [P, 32 * P], bf16, kind="ExternalInput").ap()
    a0inv_d = nc.dram_tensor("a0inv", [P, 8 * P], bf16, kind="ExternalInput").ap()
    dt_d = nc.dram_tensor("dt", [P, 2 * SC], bf16, kind="ExternalInput").ap()
    lnk_d = nc.dram_tensor("lnk", [P, 8], f32, kind="ExternalInput").ap()
    hsup_d = nc.dram_tensor("hsup", [P, 4], f32, kind="ExternalInput").ap()
    ident_d = nc.dram_tensor("ident", [P, P], bf16, kind="ExternalInput").ap()
    idx_d = nc.dram_tensor("idx16", [P, SC // 16], i16, kind="ExternalInput").ap()
    bias_d = nc.dram_tensor("biases", [P, 2], f32, kind="ExternalInput").ap()
    out_d = nc.dram_tensor("out_stats", [P, 4 * NCH], f32, kind="ExternalOutput").ap()

    with tile.TileContext(nc) as tc:
        with (
            tc.tile_pool(name="const", bufs=1) as cpool,
            tc.tile_pool(name="gat", bufs=2) as gpool,
            tc.tile_pool(name="work", bufs=1) as wpool,
            tc.tile_pool(name="small", bufs=2) as spool,
            tc.tile_pool(name="qps", bufs=2, space="PSUM") as qpool,
            tc.tile_pool(name="hps", bufs=4, space="PSUM") as hpool,
            tc.tile_pool(name="rps", bufs=2, space="PSUM") as rpool,
        ):
            invev = cpool.tile_from(invev_d)
            invpt = cpool.tile_from(invpt_d)
            a0inv = cpool.tile_from(a0inv_d)
            dt = cpool.tile_from(dt_d)
            lnk = cpool.tile_from(lnk_d)
            hsup = cpool.tile_from(hsup_d)
            ident = cpool.tile_from(ident_d)
            idx16 = cpool.tile_from(idx_d)
            biases = cpool.tile_from(bias_d)
            stats = cpool.tile([P, 4 * NCH], f32, tag="stats")

            for sc in range(NCH):
                s0 = sc * CH

                g = gpool.tile([P, 16, CH], bf16, tag="g")
                nc.gpsimd.dma_gather(
                    g,
                    maux,
                    idx16[:, sc * (CH // 16) : (sc + 1) * (CH // 16)],
                    CH,
                    CH,
                    2048,
                    transpose=True,
                )

                # ---- q = D @ inv_even + net*PM[:, idx]  (8 pipe chunks) ----
                absq = wpool.tile([P, 8 * CH], f32, tag="absq")
                lnq = wpool.tile([P, 8 * CH], f32, tag="lnq")
                e_t = wpool.tile([P, 8 * CH], bf16, tag="e_t")
                hl = wpool.tile([P, 8 * CH], bf16, tag="hl")
                qps = []
                for pc in range(8):
                    qp = qpool.tile([P, CH], f32, tag="qp")
                    nc.tensor.matmul(
                        qp,
                        invev[:, (0 * 8 + pc) * P : (0 * 8 + pc + 1) * P],
                        dt[:, 0 * SC + s0 : 0 * SC + s0 + CH],
                        start=True,
                        stop=False,
                    )
                    nc.tensor.matmul(
                        qp,
                        invev[:, (1 * 8 + pc) * P : (1 * 8 + pc + 1) * P],
                        dt[:, 1 * SC + s0 : 1 * SC + s0 + CH],
                        start=False,
                        stop=False,
                    )
                    nc.tensor.matmul(qp, ident, g[:, 4 + pc, :], start=False, stop=True)
                    sl = slice(pc * CH, (pc + 1) * CH)
                    if pc % 2 == 0:
                        nc.scalar.activation(absq[:, sl], qp, AF.Abs)
                    else:
                        # |q| on DVE: clear the sign bit on an int32 view
                        nc.vector.tensor_scalar(
                            absq[:, sl].bitcast(mybir.dt.int32),
                            qp.bitcast(mybir.dt.int32),
                            0x7FFFFFFF,
                            None,
                            OP.bitwise_and,
                        )
                    nc.scalar.activation(
                        lnq[:, sl], absq[:, sl], AF.Ln, bias=biases[:, 0:1]
                    )
                    nc.scalar.activation(
                        e_t[:, sl], lnq[:, sl], AF.Exp, bias=lnk[:, pc : pc + 1], scale=0.852
                    )
                    # hL = q * K|q|^0.852 — last reader of the q PSUM bank
                    nc.vector.tensor_tensor(hl[:, sl], qp, e_t[:, sl], OP.mult)
                    qps.append(qp)

                # ---- H = hsup - hL @ inv'^T ; sq = c0*sqrt(relu(H)) ----
                rl = wpool.tile([P, 4 * CH], bf16, tag="rl")
                lnh = wpool.tile([P, 4 * CH], f32, tag="lnh")
                sq = wpool.tile([P, 4 * CH], bf16, tag="sq")
                for n_ in range(4):
                    hp = hpool.tile([P, CH], f32, tag="hp")
                    for kc in range(8):
                        nc.tensor.matmul(
                            hp,
                            invpt[:, (kc * 4 + n_) * P : (kc * 4 + n_ + 1) * P],
                            hl[:, kc * CH : (kc + 1) * CH],
                            start=(kc == 0),
                            stop=(kc == 7),
                        )
                    nc.scalar.activation(
                        rl[:, n_ * CH : (n_ + 1) * CH],
                        hp,
                        AF.Relu,
                        bias=hsup[:, n_ : n_ + 1],
                        scale=-1.0,
                    )
                nc.scalar.activation(lnh, rl, AF.Ln, bias=biases[:, 0:1])
                nc.scalar.activation(sq, lnh, AF.Exp, scale=0.5, bias=biases[:, 1:2])

                # ---- residual chunks + sum of squares ----
                for n_ in range(4):
                    rp = rpool.tile([P, CH], f32, tag="rp")
                    nc.tensor.matmul(
                        rp,
                        a0inv[:, (0 * 4 + n_) * P : (0 * 4 + n_ + 1) * P],
                        dt[:, 0 * SC + s0 : 0 * SC + s0 + CH],
                        start=True,
                        stop=False,
                    )
                    nc.tensor.matmul(
                        rp,
                        a0inv[:, (1 * 4 + n_) * P : (1 * 4 + n_ + 1) * P],
                        dt[:, 1 * SC + s0 : 1 * SC + s0 + CH],
                        start=False,
                        stop=False,
                    )
                    nc.tensor.matmul(rp, ident, g[:, 12 + n_, :], start=False, stop=True)
                    nsl = slice(n_ * CH, (n_ + 1) * CH)
                    dl = spool.tile([P, CH], bf16, tag="dl")
                    nc.vector.tensor_tensor(dl, g[:, n_, :], sq[:, nsl], OP.mult)
                    r1 = spool.tile([P, CH], f32, tag="r1")
                    nc.vector.tensor_tensor(r1, rp, dl, OP.subtract)
                    if n_ < 2:
                        r2 = spool.tile([P, CH], f32, tag="r2")
                        nc.vector.tensor_tensor(
                            r2, r1, dt[:, n_ * SC + s0 : n_ * SC + s0 + CH], OP.subtract
                        )
                        rfin = r2
                    else:
                        rfin = r1
                    scr = spool.tile([P, CH], bf16, tag="scr")
                    nc.scalar.activation(
                        scr,
                        rfin,
                        AF.Square,
                        accum_out=stats[:, sc * 4 + n_ : sc * 4 + n_ + 1],
                    )

            nc.sync.dma_start(out_d, stats)

    nc.compile()
    return nc


def _host_prep(inputs):
    D = np.ascontiguousarray(np.asarray(inputs["D"], np.float32))
    leak = np.asarray(inputs["leak_id"]).reshape(-1).astype(np.int64)
    A0 = np.asarray(inputs["A0"], np.float32)
    inv = np.asarray(inputs["inv"], np.float32)
    M = np.asarray(inputs["M"], np.float32)
    supply = np.asarray(inputs["supply"], np.float32)
    L = np.asarray(inputs["L"], np.float32)
    d = np.asarray(inputs["d"], np.float32)
    C = np.asarray(inputs["C"], np.float32)
    a = float(np.asarray(inputs["a"]))
    Cd = float(np.asarray(inputs["Cd"]))
    W1 = np.asarray(inputs["W1"], np.float32)
    b1 = np.asarray(inputs["b1"], np.float32)
    W2 = np.asarray(inputs["W2"], np.float32)
    b2 = np.asarray(inputs["b2"], np.float32)
    W3 = np.asarray(inputs["W3"], np.float32)
    b3 = np.asarray(inputs["b3"], np.float32)
    base = np.asarray(inputs["base"], np.float32)

    # per-pipe net table (memoized MLP over the 1024 possible leak ids)
    ids = np.arange(N_PIPES, dtype=np.float32)[:, None]
    h = np.tanh(ids @ W1 + b1)
    h = np.tanh(h @ W2 + b2)
    table = base + (h @ W3 + b3)[:, 0]

    perm = np.concatenate([np.arange(0, N_NODES, 2), np.arange(1, N_NODES, 2)])
    Mp = M[perm]
    invp = inv[perm]
    inv_ev = invp[:N_DEM]  # rows of inv at even node indices

    PM = inv.T @ M                      # [1024p, 1024t]
    PMn = PM * table[None, :]
    A0p = A0[perm]
    AMn = (A0p @ PM) * table[None, :]   # [512n, 1024t]
    A0inv = A0p @ inv_ev.T              # [512n, 256j]

    maux = np.concatenate([Mp.T, PMn.T, AMn.T], axis=1).astype(BF16)  # [1024, 2048]

    def blocks(mat, kb, mb):
        # [kb*128, mb*128] -> [128, kb*mb*128], block b = kc*mb + mc
        out = np.empty((P, kb * mb * P), np.float32)
        for kc in range(kb):
            for mc in range(mb):
                b = kc * mb + mc
                out[:, b * P : (b + 1) * P] = mat[
                    kc * P : (kc + 1) * P, mc * P : (mc + 1) * P
                ]
        return out

    invev_l = blocks(inv_ev, 2, 8).astype(BF16)
    invpt_l = blocks(invp.T, 8, 4).astype(BF16)
    a0inv_l = blocks(A0inv.T, 2, 4).astype(BF16)

    K = 10.667 * C**-1.852 * d**-4.871 * L
    lnk_l = np.ascontiguousarray(np.log(K).reshape(8, P).T).astype(np.float32)
    hsup_l = np.ascontiguousarray((invp @ supply).reshape(4, P).T).astype(np.float32)
    ident = np.eye(P, dtype=np.float32).astype(BF16)
    c0 = Cd * a * math.sqrt(2.0 * G_ACC)

    dts = []
    idxs = []
    for c in range(N_CORES):
        Dc = D[c * SC : (c + 1) * SC]  # [2048, 256]
        DT = np.ascontiguousarray(Dc.T).astype(BF16)  # [256, 2048]
        dts.append(np.concatenate([DT[:P], DT[P:]], axis=1))  # [128, 4096]
        w16 = np.zeros((16, SC // 16), np.int16)
        lc = leak[c * SC : (c + 1) * SC]
        for sc in range(NCH):
            w16[:, sc * (CH // 16) : (sc + 1) * (CH // 16)] = (
                lc[sc * CH : (sc + 1) * CH].reshape(CH // 16, 16).T
            )
        # the gather firmware's Q7 cores read the index block from their own
        # 16-partition group — replicate it across all 8 groups
        idxs.append(np.tile(w16, (8, 1)))

    shared = {
        "maux": maux,
        "invev": invev_l,
        "invpt": invpt_l,
        "a0inv": a0inv_l,
        "lnk": lnk_l,
        "hsup": hsup_l,
        "ident": ident,
    }
    return shared, dts, idxs, c0


LAST_RESULTS = None


def kernel(**inputs) -> np.ndarray:
    global LAST_RESULTS
    from concourse.bass_utils import run_bass_kernel_spmd

    shared, dts, idxs, c0 = _host_prep(inputs)

    if "nc" not in _MODULE_CACHE:
        _MODULE_CACHE["nc"] = _build_module()
    nc = _MODULE_CACHE["nc"]
    bias_arr = np.zeros((P, 2), np.float32)
    bias_arr[:, 0] = 1e-35
    bias_arr[:, 1] = math.log(c0)

    in_maps = []
    for c in range(N_CORES):
        m = dict(shared)
        m["biases"] = bias_arr
        m["dt"] = dts[c]
        m["idx16"] = idxs[c]
        in_maps.append(m)

    import os

    res = run_bass_kernel_spmd(
        nc,
        in_maps,
        core_ids=list(range(N_CORES)),
        trace=bool(os.environ.get("BASS_TRACE")),
    )
    LAST_RESULTS = res

    total = 0.0
    for r in res.results:
        total += float(r["out_stats"].astype(np.float64).sum())
    return np.float32(total / (S_TOTAL * N_NODES))


# revision 10
# speedup vs baseline: 1.2117x; 1.2117x over previous
"""Trainium2 Bass kernel for the water-network leak MSE model.

Math (reference):
    net(s)   = base[idx_s] + MLP(idx_s)                    (idx_s in [0,1024))
    y        = net*onehot(idx) @ M^T + demand              demand[:, 2j] = D[:, j]
    q        = y @ inv
    hL       = sign(q) * K * |q|^1.852,  K = 10.667 C^-1.852 d^-4.871 L
    H        = (supply - hL) @ inv^T
    d_leak   = Cd*a*sqrt(2g) * (onehot @ M^T) * sqrt(relu(H))
    out      = mean((q @ A0^T - demand - d_leak)^2)

Device strategy (8 cores, data-parallel over samples, 2048 samples/core):
  All sample-independent weight transforms are folded on the host:
    PM  = inv^T M   (so q = net * PM[:, idx] + D @ inv_even),
    AM  = A0' PM    (so q @ A0'^T = net * AM[:, idx] + D @ (A0' inv_even^T)^T),
  with the per-pipe net table pre-multiplied into PM/AM columns, and the
  Hazen-Williams coefficient folded into q itself (q' = K^{1/1.852} q, so
  hL = q'|q'|^0.852 needs no per-pipe scaling on device). Node rows are
  permuted even-first so the demand subtraction is a contiguous slice.
  On device, per 512-sample chunk (features on partitions, samples on free):
    - one transposed dma_gather pulls M^T/PM^T/AM^T columns for the chunk's
      leak ids directly into [feature, sample] layout (bf16),
    - PE: D-part matmuls (K=256) for q and the residual, identity-matmul
      injects of the gathered parts into PSUM, and the full H matmul (K=1024),
    - ACT: ln/exp power chains (natural_log_exp table set only, loaded once),
    - DVE: |q| (sign-bit clear), hL = q*e from PSUM, residual assembly,
      fused square+reduce partials,
    - Pool: gathers and d_leak elementwise.
  q is processed in two 4-bank PSUM waves so hL reads PSUM directly and the
  banks recycle (PSUM budget: 4 q + 2 H + 2 R = 8 banks).
  Each core returns [128, 16] partial sums of squares; host reduces.
"""

import math

import numpy as np
import ml_dtypes

P = 128
N_CORES = 8
S_TOTAL = 16384
SC = S_TOTAL // N_CORES  # samples per core
CH = 512                 # samples per chunk
NCH = SC // CH           # chunks per core
N_NODES = 512
N_PIPES = 1024
N_DEM = 256
G_ACC = 9.80665

BF16 = ml_dtypes.bfloat16

_MODULE_CACHE: dict = {}


def _build_module():
    import concourse.bacc as bacc
    import concourse.mybir as mybir
    import concourse.tile as tile

    f32 = mybir.dt.float32
    bf16 = mybir.dt.bfloat16
    i16 = mybir.dt.int16
    AF = mybir.ActivationFunctionType
    OP = mybir.AluOpType

    nc = bacc.Bacc(trn_type="TRN2", target_bir_lowering=False, debug=False)

    maux = nc.dram_tensor("maux", [N_PIPES, 2048], bf16, kind="ExternalInput").ap()
    invev_d = nc.dram_tensor("invev", [P, 16 * P], bf16, kind="ExternalInput").ap()
    invpt_d = nc.dram_tensor("invpt", [P, 32 * P], bf16, kind="ExternalInput").ap()
    a0inv_d = nc.dram_tensor("a0inv", [P, 8 * P], bf16, kind="ExternalInput").ap()
    dt_d = nc.dram_tensor("dt", [P, 2 * SC], bf16, kind="ExternalInput").ap()
    hsup_d = nc.dram_tensor("hsup", [P, 4], f32, kind="ExternalInput").ap()
    ident_d = nc.dram_tensor("ident", [P, P], bf16, kind="ExternalInput").ap()
    idx_d = nc.dram_tensor("idx16", [P, SC // 16], i16, kind="ExternalInput").ap()
    bias_d = nc.dram_tensor("biases", [P, 2], f32, kind="ExternalInput").ap()
    out_d = nc.dram_tensor("out_stats", [P, 4 * NCH], f32, kind="ExternalOutput").ap()

    with tile.TileContext(nc) as tc:
        with (
            tc.tile_pool(name="const", bufs=1) as cpool,
            tc.tile_pool(name="gat", bufs=3) as gpool,
            tc.tile_pool(name="work", bufs=1) as wpool,
            tc.tile_pool(name="small", bufs=2) as spool,
            tc.tile_pool(name="qps", bufs=4, space="PSUM") as qpool,
            tc.tile_pool(name="hps", bufs=2, space="PSUM") as hpool,
            tc.tile_pool(name="rps", bufs=2, space="PSUM") as rpool,
        ):
            invev = cpool.tile_from(invev_d)
            invpt = cpool.tile_from(invpt_d)
            a0inv = cpool.tile_from(a0inv_d)
            dt = cpool.tile_from(dt_d)
            hsup = cpool.tile_from(hsup_d)
            ident = cpool.tile_from(ident_d)
            idx16 = cpool.tile_from(idx_d)
            biases = cpool.tile_from(bias_d)
            stats = cpool.tile([P, 4 * NCH], f32, tag="stats")

            for sc in range(NCH):
                s0 = sc * CH

                g = gpool.tile([P, 16, CH], bf16, tag="g")
                nc.gpsimd.dma_gather(
                    g,
                    maux,
                    idx16[:, sc * (CH // 16) : (sc + 1) * (CH // 16)],
                    CH,
                    CH,
                    2048,
                    transpose=True,
                )

                # ---- q' = K^(1/1.852)*(D @ inv_even + net*PM[:, idx]) ----
                absq = wpool.tile([P, 8 * CH], f32, tag="absq")
                lne = wpool.tile([P, 8 * CH], f32, tag="lne")
                e_t = wpool.tile([P, 8 * CH], bf16, tag="e_t")
                hl = wpool.tile([P, 8 * CH], bf16, tag="hl", bufs=2)
                for w in range(2):
                    qps = []
                    for pc in range(4 * w, 4 * w + 4):
                        qp = qpool.tile([P, CH], f32, tag="qp")
                        nc.tensor.matmul(
                            qp,
                            invev[:, (0 * 8 + pc) * P : (0 * 8 + pc + 1) * P],
                            dt[:, 0 * SC + s0 : 0 * SC + s0 + CH],
                            start=True,
                            stop=False,
                        )
                        nc.tensor.matmul(
                            qp,
                            invev[:, (1 * 8 + pc) * P : (1 * 8 + pc + 1) * P],
                            dt[:, 1 * SC + s0 : 1 * SC + s0 + CH],
                            start=False,
                            stop=False,
                        )
                        nc.tensor.matmul(
                            qp, ident, g[:, 4 + pc, :], start=False, stop=True
                        )
                        sl = slice(pc * CH, (pc + 1) * CH)
                        if pc % 2 == 0:
                            nc.scalar.activation(absq[:, sl], qp, AF.Abs)
                        else:
                            # |q| on DVE: clear the sign bit on an int32 view
                            nc.vector.tensor_scalar(
                                absq[:, sl].bitcast(mybir.dt.int32),
                                qp.bitcast(mybir.dt.int32),
                                0x7FFFFFFF,
                                None,
                                OP.bitwise_and,
                            )
                        qps.append(qp)
                    wsl = slice(w * 4 * CH, (w + 1) * 4 * CH)
                    nc.scalar.activation(
                        lne[:, wsl], absq[:, wsl], AF.Ln, bias=biases[:, 0:1]
                    )
                    nc.scalar.activation(e_t[:, wsl], lne[:, wsl], AF.Exp, scale=0.852)
                    for i, pc in enumerate(range(4 * w, 4 * w + 4)):
                        sl = slice(pc * CH, (pc + 1) * CH)
                        # hL = q'|q'|^0.852 — last reader of the q PSUM bank
                        nc.vector.tensor_tensor(hl[:, sl], qps[i], e_t[:, sl], OP.mult)

                # ---- H = hsup - hL @ inv'^T ; sq = c0*sqrt(relu(H)) ----
                rl = wpool.tile([P, 4 * CH], bf16, tag="rl", bufs=2)
                lnh = wpool.tile([P, 4 * CH], f32, tag="lnh")
                sq = wpool.tile([P, 4 * CH], bf16, tag="sq", bufs=2)
                for n_ in range(4):
                    hp = hpool.tile([P, CH], f32, tag="hp")
                    for kc in range(8):
                        nc.tensor.matmul(
                            hp,
                            invpt[:, (kc * 4 + n_) * P : (kc * 4 + n_ + 1) * P],
                            hl[:, kc * CH : (kc + 1) * CH],
                            start=(kc == 0),
                            stop=(kc == 7),
                        )
                    nc.scalar.activation(
                        rl[:, n_ * CH : (n_ + 1) * CH],
                        hp,
                        AF.Relu,
                        bias=hsup[:, n_ : n_ + 1],
                        scale=-1.0,
                    )
                nc.scalar.activation(lnh, rl, AF.Ln, bias=biases[:, 0:1])
                nc.scalar.activation(sq, lnh, AF.Exp, scale=0.5, bias=biases[:, 1:2])

                # ---- residual chunks + sum of squares ----
                for n_ in range(4):
                    rp = rpool.tile([P, CH], f32, tag="rp")
                    nc.tensor.matmul(
                        rp,
                        a0inv[:, (0 * 4 + n_) * P : (0 * 4 + n_ + 1) * P],
                        dt[:, 0 * SC + s0 : 0 * SC + s0 + CH],
                        start=True,
                        stop=False,
                    )
                    nc.tensor.matmul(
                        rp,
                        a0inv[:, (1 * 4 + n_) * P : (1 * 4 + n_ + 1) * P],
                        dt[:, 1 * SC + s0 : 1 * SC + s0 + CH],
                        start=False,
                        stop=False,
                    )
                    nc.tensor.matmul(rp, ident, g[:, 12 + n_, :], start=False, stop=True)
                    nsl = slice(n_ * CH, (n_ + 1) * CH)
                    dl = spool.tile([P, CH], bf16, tag="dl")
                    nc.vector.tensor_tensor(dl, g[:, n_, :], sq[:, nsl], OP.mult)
                    r1 = spool.tile([P, CH], f32, tag="r1")
                    nc.vector.tensor_tensor(r1, rp, dl, OP.subtract)
                    if n_ < 2:
                        r2 = spool.tile([P, CH], f32, tag="r2")
                        nc.vector.tensor_tensor(
                            r2, r1, dt[:, n_ * SC + s0 : n_ * SC + s0 + CH], OP.subtract
                        )
                        rfin = r2
                    else:
                        rfin = r1
                    scr = spool.tile([P, CH], bf16, tag="scr")
                    nc.scalar.activation(
                        scr,
                        rfin,
                        AF.Square,
                        accum_out=stats[:, sc * 4 + n_ : sc * 4 + n_ + 1],
                    )

            nc.sync.dma_start(out_d, stats)

    nc.compile()
    return nc


def _host_prep(inputs):
    D = np.ascontiguousarray(np.asarray(inputs["D"], np.float32))
    leak = np.asarray(inputs["leak_id"]).reshape(-1).astype(np.int64)
    A0 = np.asarray(inputs["A0"], np.float32)
    inv = np.asarray(inputs["inv"], np.float32)
    M = np.asarray(inputs["M"], np.float32)
    supply = np.asarray(inputs["supply"], np.float32)
    L = np.asarray(inputs["L"], np.float32)
    d = np.asarray(inputs["d"], np.float32)
    C = np.asarray(inputs["C"], np.float32)
    a = float(np.asarray(inputs["a"]))
    Cd = float(np.asarray(inputs["Cd"]))
    W1 = np.asarray(inputs["W1"], np.float32)
    b1 = np.asarray(inputs["b1"], np.float32)
    W2 = np.asarray(inputs["W2"], np.float32)
    b2 = np.asarray(inputs["b2"], np.float32)
    W3 = np.asarray(inputs["W3"], np.float32)
    b3 = np.asarray(inputs["b3"], np.float32)
    base = np.asarray(inputs["base"], np.float32)

    # per-pipe net table (memoized MLP over the 1024 possible leak ids)
    ids = np.arange(N_PIPES, dtype=np.float32)[:, None]
    h = np.tanh(ids @ W1 + b1)
    h = np.tanh(h @ W2 + b2)
    table = base + (h @ W3 + b3)[:, 0]

    perm = np.concatenate([np.arange(0, N_NODES, 2), np.arange(1, N_NODES, 2)])
    Mp = M[perm]
    invp = inv[perm]
    inv_ev = invp[:N_DEM]  # rows of inv at even node indices

    K = 10.667 * C**-1.852 * d**-4.871 * L
    k1 = K ** (1.0 / 1.852)  # fold into q so hL = q'|q'|^0.852

    PM = inv.T @ M                        # [1024p, 1024t]
    PMn = (PM * table[None, :]) * k1[:, None]
    A0p = A0[perm]
    AMn = (A0p @ PM) * table[None, :]     # [512n, 1024t]
    A0inv = A0p @ inv_ev.T                # [512n, 256j]

    maux = np.concatenate([Mp.T, PMn.T, AMn.T], axis=1).astype(BF16)  # [1024, 2048]

    def blocks(mat, kb, mb):
        # [kb*128, mb*128] -> [128, kb*mb*128], block b = kc*mb + mc
        out = np.empty((P, kb * mb * P), np.float32)
        for kc in range(kb):
            for mc in range(mb):
                b = kc * mb + mc
                out[:, b * P : (b + 1) * P] = mat[
                    kc * P : (kc + 1) * P, mc * P : (mc + 1) * P
                ]
        return out

    invev_l = blocks(inv_ev * k1[None, :], 2, 8).astype(BF16)
    invpt_l = blocks(invp.T, 8, 4).astype(BF16)
    a0inv_l = blocks(A0inv.T, 2, 4).astype(BF16)

    hsup_l = np.ascontiguousarray((invp @ supply).reshape(4, P).T).astype(np.float32)
    ident = np.eye(P, dtype=np.float32).astype(BF16)
    c0 = Cd * a * math.sqrt(2.0 * G_ACC)

    dts = []
    idxs = []
    for c in range(N_CORES):
        Dc = D[c * SC : (c + 1) * SC]  # [2048, 256]
        DT = np.ascontiguousarray(Dc.T).astype(BF16)  # [256, 2048]
        dts.append(np.concatenate([DT[:P], DT[P:]], axis=1))  # [128, 4096]
        w16 = np.zeros((16, SC // 16), np.int16)
        lc = leak[c * SC : (c + 1) * SC]
        for sc in range(NCH):
            w16[:, sc * (CH // 16) : (sc + 1) * (CH // 16)] = (
                lc[sc * CH : (sc + 1) * CH].reshape(CH // 16, 16).T
            )
        # the gather firmware's Q7 cores read the index block from their own
        # 16-partition group — replicate it across all 8 groups
        idxs.append(np.tile(w16, (8, 1)))

    shared = {
        "maux": maux,
        "invev": invev_l,
        "invpt": invpt_l,
        "a0inv": a0inv_l,
        "hsup": hsup_l,
        "ident": ident,
    }
    return shared, dts, idxs, c0


LAST_RESULTS = None


def kernel(**inputs) -> np.ndarray:
    global LAST_RESULTS
    from concourse.bass_utils import run_bass_kernel_spmd

    shared, dts, idxs, c0 = _host_prep(inputs)

    if "nc" not in _MODULE_CACHE:
        _MODULE_CACHE["nc"] = _build_module()
    nc = _MODULE_CACHE["nc"]
    bias_arr = np.zeros((P, 2), np.float32)
    bias_arr[:, 0] = 1e-35
    bias_arr[:, 1] = math.log(c0)

    in_maps = []
    for c in range(N_CORES):
        m = dict(shared)
        m["biases"] = bias_arr
        m["dt"] = dts[c]
        m["idx16"] = idxs[c]
        in_maps.append(m)

    import os

    res = run_bass_kernel_spmd(
        nc,
        in_maps,
        core_ids=list(range(N_CORES)),
        trace=bool(os.environ.get("BASS_TRACE")),
    )
    LAST_RESULTS = res

    total = 0.0
    for r in res.results:
        total += float(r["out_stats"].astype(np.float64).sum())
    return np.float32(total / (S_TOTAL * N_NODES))


# revision 11
# speedup vs baseline: 1.5877x; 1.3103x over previous
"""Trainium2 Bass kernel for the water-network leak MSE model.

Math (reference):
    net(s)   = base[idx_s] + MLP(idx_s)                    (idx_s in [0,1024))
    y        = net*onehot(idx) @ M^T + demand              demand[:, 2j] = D[:, j]
    q        = y @ inv
    hL       = sign(q) * K * |q|^1.852,  K = 10.667 C^-1.852 d^-4.871 L
    H        = (supply - hL) @ inv^T
    d_leak   = Cd*a*sqrt(2g) * (onehot @ M^T) * sqrt(relu(H))
    out      = mean((q @ A0^T - demand - d_leak)^2)

Device strategy (8 cores, data-parallel over samples, 2048 samples/core):
  All sample-independent weight transforms are folded on the host:
    PM  = inv^T M   (so q = net * PM[:, idx] + D @ inv_even),
    AM  = A0' PM    (so q @ A0'^T = net * AM[:, idx] + D @ (A0' inv_even^T)^T),
  with the per-pipe net table pre-multiplied into PM/AM columns, and the
  Hazen-Williams coefficient folded into q itself (q' = K^{1/1.852} q, so
  hL = q'|q'|^0.852 needs no per-pipe scaling on device). Node rows are
  permuted even-first so the demand subtraction is a contiguous slice.
  On device, per 512-sample chunk (features on partitions, samples on free):
    - one transposed dma_gather pulls M^T/PM^T/AM^T columns for the chunk's
      leak ids directly into [feature, sample] layout (bf16),
    - PE: D-part matmuls (K=256) for q and the residual, identity-matmul
      injects of the gathered parts into PSUM, and the full H matmul (K=1024),
    - ACT: ln/exp power chains (natural_log_exp table set only, loaded once),
    - DVE: |q| (sign-bit clear), hL = q*e from PSUM, residual assembly,
      fused square+reduce partials,
    - Pool: gathers and d_leak elementwise.
  q is processed in two 4-bank PSUM waves so hL reads PSUM directly and the
  banks recycle (PSUM budget: 4 q + 2 H + 2 R = 8 banks).
  Each core returns [128, 16] partial sums of squares; host reduces.
"""

import math

import numpy as np
import ml_dtypes

P = 128
N_CORES = 8
S_TOTAL = 16384
SC = S_TOTAL // N_CORES  # samples per core
CH = 512                 # samples per chunk
NCH = SC // CH           # chunks per core
N_NODES = 512
N_PIPES = 1024
N_DEM = 256
G_ACC = 9.80665

BF16 = ml_dtypes.bfloat16

_MODULE_CACHE: dict = {}


def _build_module():
    import concourse.bacc as bacc
    import concourse.mybir as mybir
    import concourse.tile as tile

    f32 = mybir.dt.float32
    bf16 = mybir.dt.bfloat16
    i16 = mybir.dt.int16
    AF = mybir.ActivationFunctionType
    OP = mybir.AluOpType

    nc = bacc.Bacc(trn_type="TRN2", target_bir_lowering=False, debug=False)

    # All our activations (Abs/Relu/Square/Ln/Exp) live in the
    # natural_log_exp_and_others table set, but the table-load pass maps each
    # func to the first set containing it, ping-ponging between exp_and_others
    # and natural_log (25 table loads, ~40us of ACT). Strip our funcs from
    # every other set so the pass converges on the one shared set.
    import types as _types
    from concourse.hw_specs import get_activation_tables as _gat
    import bass_rust as _bass_rust

    _OURS = {AF.Abs, AF.Relu, AF.Square, AF.Ln, AF.Exp, AF.Identity, AF.Copy,
             AF.Sign, AF.MemsetZero}

    def _patched_act_table_loads(self):
        has_activation = any(
            isinstance(i, mybir.InstActivation)
            for b in self.main_func.blocks
            for i in b.instructions
        )
        if not has_activation:
            return
        tables = []
        for name, fns in _gat(self.m.arch).items():
            if name != "natural_log_exp_and_others":
                fns = fns - _OURS
            tables.append((name, fns))
        _bass_rust.insert_act_table_loads(self, tables)

    nc.insert_act_table_loads = _types.MethodType(_patched_act_table_loads, nc)

    maux = nc.dram_tensor("maux", [N_PIPES, 2048], bf16, kind="ExternalInput").ap()
    invev_d = nc.dram_tensor("invev", [P, 16 * P], bf16, kind="ExternalInput").ap()
    invpt_d = nc.dram_tensor("invpt", [P, 32 * P], bf16, kind="ExternalInput").ap()
    a0inv_d = nc.dram_tensor("a0inv", [P, 8 * P], bf16, kind="ExternalInput").ap()
    dt_d = nc.dram_tensor("dt", [P, 2 * SC], bf16, kind="ExternalInput").ap()
    hsup_d = nc.dram_tensor("hsup", [P, 4], f32, kind="ExternalInput").ap()
    ident_d = nc.dram_tensor("ident", [P, P], bf16, kind="ExternalInput").ap()
    idx_d = nc.dram_tensor("idx16", [P, SC // 16], i16, kind="ExternalInput").ap()
    bias_d = nc.dram_tensor("biases", [P, 2], f32, kind="ExternalInput").ap()
    out_d = nc.dram_tensor("out_stats", [P, NCH], f32, kind="ExternalOutput").ap()

    with tile.TileContext(nc) as tc:
        with (
            tc.tile_pool(name="const", bufs=1) as cpool,
            tc.tile_pool(name="gat", bufs=3) as gpool,
            tc.tile_pool(name="work", bufs=1) as wpool,
            tc.tile_pool(name="small", bufs=2) as spool,
            tc.tile_pool(name="qps", bufs=4, space="PSUM") as qpool,
            tc.tile_pool(name="hps", bufs=2, space="PSUM") as hpool,
            tc.tile_pool(name="rps", bufs=2, space="PSUM") as rpool,
        ):
            invev = cpool.tile_from(invev_d)
            invpt = cpool.tile_from(invpt_d)
            a0inv = cpool.tile_from(a0inv_d)
            dt = cpool.tile_from(dt_d)
            hsup = cpool.tile_from(hsup_d)
            ident = cpool.tile_from(ident_d)
            idx16 = cpool.tile_from(idx_d)
            biases = cpool.tile_from(bias_d)
            stats = cpool.tile([P, NCH], f32, tag="stats")

            for sc in range(NCH):
                s0 = sc * CH

                g = gpool.tile([P, 16, CH], bf16, tag="g")
                nc.gpsimd.dma_gather(
                    g,
                    maux,
                    idx16[:, sc * (CH // 16) : (sc + 1) * (CH // 16)],
                    CH,
                    CH,
                    2048,
                    transpose=True,
                )

                # ---- q' = K^(1/1.852)*(D @ inv_even + net*PM[:, idx]) ----
                # 4 waves of 2 pipe-chunks: 2 waves in flight in PSUM so PE
                # matmuls of wave w+1 overlap the ACT/DVE chain of wave w.
                hl = wpool.tile([P, 8 * CH], bf16, tag="hl", bufs=2)
                for w in range(4):
                    absq = wpool.tile([P, 2 * CH], f32, tag="absq", bufs=3)
                    lne = wpool.tile([P, 2 * CH], f32, tag="lne", bufs=2)
                    e_t = wpool.tile([P, 2 * CH], bf16, tag="e_t", bufs=3)
                    qps = []
                    for i, pc in enumerate(range(2 * w, 2 * w + 2)):
                        qp = qpool.tile([P, CH], f32, tag="qp")
                        nc.tensor.matmul(
                            qp,
                            invev[:, (0 * 8 + pc) * P : (0 * 8 + pc + 1) * P],
                            dt[:, 0 * SC + s0 : 0 * SC + s0 + CH],
                            start=True,
                            stop=False,
                        )
                        nc.tensor.matmul(
                            qp,
                            invev[:, (1 * 8 + pc) * P : (1 * 8 + pc + 1) * P],
                            dt[:, 1 * SC + s0 : 1 * SC + s0 + CH],
                            start=False,
                            stop=False,
                        )
                        nc.tensor.matmul(
                            qp, ident, g[:, 4 + pc, :], start=False, stop=True
                        )
                        # |q| on DVE: clear the sign bit on an int32 view
                        nc.vector.tensor_scalar(
                            absq[:, i * CH : (i + 1) * CH].bitcast(mybir.dt.int32),
                            qp.bitcast(mybir.dt.int32),
                            0x7FFFFFFF,
                            None,
                            OP.bitwise_and,
                        )
                        qps.append(qp)
                    nc.scalar.activation(lne, absq, AF.Ln, bias=biases[:, 0:1])
                    nc.scalar.activation(e_t, lne, AF.Exp, scale=0.852)
                    for i, pc in enumerate(range(2 * w, 2 * w + 2)):
                        # hL = q'|q'|^0.852 — last reader of the q PSUM bank
                        nc.vector.tensor_tensor(
                            hl[:, pc * CH : (pc + 1) * CH],
                            qps[i],
                            e_t[:, i * CH : (i + 1) * CH],
                            OP.mult,
                        )

                # ---- H = hsup - hL @ inv'^T ; sq = c0*sqrt(relu(H)) ----
                rl = wpool.tile([P, 4 * CH], bf16, tag="rl", bufs=2)
                lnh = wpool.tile([P, 4 * CH], f32, tag="lnh")
                sq = wpool.tile([P, 4 * CH], bf16, tag="sq", bufs=2)
                for n_ in range(4):
                    hp = hpool.tile([P, CH], f32, tag="hp")
                    for kc in range(8):
                        nc.tensor.matmul(
                            hp,
                            invpt[:, (kc * 4 + n_) * P : (kc * 4 + n_ + 1) * P],
                            hl[:, kc * CH : (kc + 1) * CH],
                            start=(kc == 0),
                            stop=(kc == 7),
                        )
                    nc.scalar.activation(
                        rl[:, n_ * CH : (n_ + 1) * CH],
                        hp,
                        AF.Relu,
                        bias=hsup[:, n_ : n_ + 1],
                        scale=-1.0,
                    )
                nc.scalar.activation(lnh, rl, AF.Ln, bias=biases[:, 0:1])
                nc.scalar.activation(sq, lnh, AF.Exp, scale=0.5, bias=biases[:, 1:2])

                # ---- residual chunks + sum of squares ----
                r_all = wpool.tile([P, 4 * CH], f32, tag="r_all", bufs=2)
                for n_ in range(4):
                    rp = rpool.tile([P, CH], f32, tag="rp")
                    nc.tensor.matmul(
                        rp,
                        a0inv[:, (0 * 4 + n_) * P : (0 * 4 + n_ + 1) * P],
                        dt[:, 0 * SC + s0 : 0 * SC + s0 + CH],
                        start=True,
                        stop=False,
                    )
                    nc.tensor.matmul(
                        rp,
                        a0inv[:, (1 * 4 + n_) * P : (1 * 4 + n_ + 1) * P],
                        dt[:, 1 * SC + s0 : 1 * SC + s0 + CH],
                        start=False,
                        stop=False,
                    )
                    nc.tensor.matmul(rp, ident, g[:, 12 + n_, :], start=False, stop=True)
                    nsl = slice(n_ * CH, (n_ + 1) * CH)
                    dl = spool.tile([P, CH], bf16, tag="dl")
                    nc.vector.tensor_tensor(dl, g[:, n_, :], sq[:, nsl], OP.mult)
                    if n_ < 2:
                        r1 = spool.tile([P, CH], f32, tag="r1")
                        nc.vector.tensor_tensor(r1, rp, dl, OP.subtract)
                        nc.vector.tensor_tensor(
                            r_all[:, nsl],
                            r1,
                            dt[:, n_ * SC + s0 : n_ * SC + s0 + CH],
                            OP.subtract,
                        )
                    else:
                        nc.vector.tensor_tensor(r_all[:, nsl], rp, dl, OP.subtract)
                scr = wpool.tile([P, 4 * CH], bf16, tag="scr", bufs=2)
                nc.scalar.activation(
                    scr, r_all, AF.Square, accum_out=stats[:, sc : sc + 1]
                )

            nc.sync.dma_start(out_d, stats)

    nc.compile()
    return nc


def _host_prep(inputs):
    D = np.ascontiguousarray(np.asarray(inputs["D"], np.float32))
    leak = np.asarray(inputs["leak_id"]).reshape(-1).astype(np.int64)
    A0 = np.asarray(inputs["A0"], np.float32)
    inv = np.asarray(inputs["inv"], np.float32)
    M = np.asarray(inputs["M"], np.float32)
    supply = np.asarray(inputs["supply"], np.float32)
    L = np.asarray(inputs["L"], np.float32)
    d = np.asarray(inputs["d"], np.float32)
    C = np.asarray(inputs["C"], np.float32)
    a = float(np.asarray(inputs["a"]))
    Cd = float(np.asarray(inputs["Cd"]))
    W1 = np.asarray(inputs["W1"], np.float32)
    b1 = np.asarray(inputs["b1"], np.float32)
    W2 = np.asarray(inputs["W2"], np.float32)
    b2 = np.asarray(inputs["b2"], np.float32)
    W3 = np.asarray(inputs["W3"], np.float32)
    b3 = np.asarray(inputs["b3"], np.float32)
    base = np.asarray(inputs["base"], np.float32)

    # per-pipe net table (memoized MLP over the 1024 possible leak ids)
    ids = np.arange(N_PIPES, dtype=np.float32)[:, None]
    h = np.tanh(ids @ W1 + b1)
    h = np.tanh(h @ W2 + b2)
    table = base + (h @ W3 + b3)[:, 0]

    perm = np.concatenate([np.arange(0, N_NODES, 2), np.arange(1, N_NODES, 2)])
    Mp = M[perm]
    invp = inv[perm]
    inv_ev = invp[:N_DEM]  # rows of inv at even node indices

    K = 10.667 * C**-1.852 * d**-4.871 * L
    k1 = K ** (1.0 / 1.852)  # fold into q so hL = q'|q'|^0.852

    PM = inv.T @ M                        # [1024p, 1024t]
    PMn = (PM * table[None, :]) * k1[:, None]
    A0p = A0[perm]
    AMn = (A0p @ PM) * table[None, :]     # [512n, 1024t]
    A0inv = A0p @ inv_ev.T                # [512n, 256j]

    maux = np.concatenate([Mp.T, PMn.T, AMn.T], axis=1).astype(BF16)  # [1024, 2048]

    def blocks(mat, kb, mb):
        # [kb*128, mb*128] -> [128, kb*mb*128], block b = kc*mb + mc
        out = np.empty((P, kb * mb * P), np.float32)
        for kc in range(kb):
            for mc in range(mb):
                b = kc * mb + mc
                out[:, b * P : (b + 1) * P] = mat[
                    kc * P : (kc + 1) * P, mc * P : (mc + 1) * P
                ]
        return out

    invev_l = blocks(inv_ev * k1[None, :], 2, 8).astype(BF16)
    invpt_l = blocks(invp.T, 8, 4).astype(BF16)
    a0inv_l = blocks(A0inv.T, 2, 4).astype(BF16)

    hsup_l = np.ascontiguousarray((invp @ supply).reshape(4, P).T).astype(np.float32)
    ident = np.eye(P, dtype=np.float32).astype(BF16)
    c0 = Cd * a * math.sqrt(2.0 * G_ACC)

    dts = []
    idxs = []
    for c in range(N_CORES):
        Dc = D[c * SC : (c + 1) * SC]  # [2048, 256]
        DT = np.ascontiguousarray(Dc.T).astype(BF16)  # [256, 2048]
        dts.append(np.concatenate([DT[:P], DT[P:]], axis=1))  # [128, 4096]
        w16 = np.zeros((16, SC // 16), np.int16)
        lc = leak[c * SC : (c + 1) * SC]
        for sc in range(NCH):
            w16[:, sc * (CH // 16) : (sc + 1) * (CH // 16)] = (
                lc[sc * CH : (sc + 1) * CH].reshape(CH // 16, 16).T
            )
        # the gather firmware's Q7 cores read the index block from their own
        # 16-partition group — replicate it across all 8 groups
        idxs.append(np.tile(w16, (8, 1)))

    shared = {
        "maux": maux,
        "invev": invev_l,
        "invpt": invpt_l,
        "a0inv": a0inv_l,
        "hsup": hsup_l,
        "ident": ident,
    }
    return shared, dts, idxs, c0


LAST_RESULTS = None


def kernel(**inputs) -> np.ndarray:
    global LAST_RESULTS
    from concourse.bass_utils import run_bass_kernel_spmd

    shared, dts, idxs, c0 = _host_prep(inputs)

    if "nc" not in _MODULE_CACHE:
        _MODULE_CACHE["nc"] = _build_module()
    nc = _MODULE_CACHE["nc"]
    bias_arr = np.zeros((P, 2), np.float32)
    bias_arr[:, 0] = 1e-35
    bias_arr[:, 1] = math.log(c0)

    in_maps = []
    for c in range(N_CORES):
        m = dict(shared)
        m["biases"] = bias_arr
        m["dt"] = dts[c]
        m["idx16"] = idxs[c]
        in_maps.append(m)

    import os

    res = run_bass_kernel_spmd(
        nc,
        in_maps,
        core_ids=list(range(N_CORES)),
        trace=bool(os.environ.get("BASS_TRACE")),
    )
    LAST_RESULTS = res

    total = 0.0
    for r in res.results:
        total += float(r["out_stats"].astype(np.float64).sum())
    return np.float32(total / (S_TOTAL * N_NODES))


# revision 12
# speedup vs baseline: 1.6108x; 1.0145x over previous
"""Trainium2 Bass kernel for the water-network leak MSE model.

Math (reference):
    net(s)   = base[idx_s] + MLP(idx_s)                    (idx_s in [0,1024))
    y        = net*onehot(idx) @ M^T + demand              demand[:, 2j] = D[:, j]
    q        = y @ inv
    hL       = sign(q) * K * |q|^1.852,  K = 10.667 C^-1.852 d^-4.871 L
    H        = (supply - hL) @ inv^T
    d_leak   = Cd*a*sqrt(2g) * (onehot @ M^T) * sqrt(relu(H))
    out      = mean((q @ A0^T - demand - d_leak)^2)

Device strategy (8 cores, data-parallel over samples, 2048 samples/core):
  All sample-independent weight transforms are folded on the host:
    PM  = inv^T M   (so q = net * PM[:, idx] + D @ inv_even),
    AM  = A0' PM    (so q @ A0'^T = net * AM[:, idx] + D @ (A0' inv_even^T)^T),
  with the per-pipe net table pre-multiplied into PM/AM columns, and the
  Hazen-Williams coefficient folded into q itself (q' = K^{1/1.852} q, so
  hL = q'|q'|^0.852 needs no per-pipe scaling on device). Node rows are
  permuted even-first so the demand subtraction is a contiguous slice.
  On device, per 512-sample chunk (features on partitions, samples on free):
    - one transposed dma_gather pulls M^T/PM^T/AM^T columns for the chunk's
      leak ids directly into [feature, sample] layout (bf16),
    - PE: D-part matmuls (K=256) for q and the residual, identity-matmul
      injects of the gathered parts into PSUM, and the full H matmul (K=1024),
    - ACT: ln/exp power chains (natural_log_exp table set only, loaded once),
    - DVE: |q| (sign-bit clear), hL = q*e from PSUM, residual assembly,
      fused square+reduce partials,
    - Pool: gathers and d_leak elementwise.
  q is processed in two 4-bank PSUM waves so hL reads PSUM directly and the
  banks recycle (PSUM budget: 4 q + 2 H + 2 R = 8 banks).
  Each core returns [128, 16] partial sums of squares; host reduces.
"""

import math

import numpy as np
import ml_dtypes

P = 128
N_CORES = 8
S_TOTAL = 16384
SC = S_TOTAL // N_CORES  # samples per core
CH = 512                 # samples per chunk
NCH = SC // CH           # chunks per core
N_NODES = 512
N_PIPES = 1024
N_DEM = 256
G_ACC = 9.80665

BF16 = ml_dtypes.bfloat16

_MODULE_CACHE: dict = {}


def _build_module():
    import concourse.bacc as bacc
    import concourse.mybir as mybir
    import concourse.tile as tile

    f32 = mybir.dt.float32
    bf16 = mybir.dt.bfloat16
    i16 = mybir.dt.int16
    AF = mybir.ActivationFunctionType
    OP = mybir.AluOpType

    nc = bacc.Bacc(trn_type="TRN2", target_bir_lowering=False, debug=False)

    # All our activations (Abs/Relu/Square/Ln/Exp) live in the
    # natural_log_exp_and_others table set, but the table-load pass maps each
    # func to the first set containing it, ping-ponging between exp_and_others
    # and natural_log (25 table loads, ~40us of ACT). Strip our funcs from
    # every other set so the pass converges on the one shared set.
    import types as _types
    from concourse.hw_specs import get_activation_tables as _gat
    import bass_rust as _bass_rust

    _OURS = {AF.Abs, AF.Relu, AF.Square, AF.Ln, AF.Exp, AF.Identity, AF.Copy,
             AF.Sign, AF.MemsetZero}

    def _patched_act_table_loads(self):
        has_activation = any(
            isinstance(i, mybir.InstActivation)
            for b in self.main_func.blocks
            for i in b.instructions
        )
        if not has_activation:
            return
        tables = []
        for name, fns in _gat(self.m.arch).items():
            if name != "natural_log_exp_and_others":
                fns = fns - _OURS
            tables.append((name, fns))
        _bass_rust.insert_act_table_loads(self, tables)

    nc.insert_act_table_loads = _types.MethodType(_patched_act_table_loads, nc)

    maux = nc.dram_tensor("maux", [N_PIPES, 2048], bf16, kind="ExternalInput").ap()
    invev_d = nc.dram_tensor("invev", [P, 16 * P], bf16, kind="ExternalInput").ap()
    invpt_d = nc.dram_tensor("invpt", [P, 32 * P], bf16, kind="ExternalInput").ap()
    a0inv_d = nc.dram_tensor("a0inv", [P, 8 * P], bf16, kind="ExternalInput").ap()
    dt_d = nc.dram_tensor("dt", [P, 2 * SC], bf16, kind="ExternalInput").ap()
    hsup_d = nc.dram_tensor("hsup", [P, 4], f32, kind="ExternalInput").ap()
    ident_d = nc.dram_tensor("ident", [P, P], bf16, kind="ExternalInput").ap()
    nident_d = nc.dram_tensor("nident", [P, P], bf16, kind="ExternalInput").ap()
    idx_d = nc.dram_tensor("idx16", [P, SC // 16], i16, kind="ExternalInput").ap()
    bias_d = nc.dram_tensor("biases", [P, 2], f32, kind="ExternalInput").ap()
    out_d = nc.dram_tensor("out_stats", [P, NCH], f32, kind="ExternalOutput").ap()

    with tile.TileContext(nc) as tc:
        with (
            tc.tile_pool(name="const", bufs=1) as cpool,
            tc.tile_pool(name="gat", bufs=3) as gpool,
            tc.tile_pool(name="work", bufs=1) as wpool,
            tc.tile_pool(name="small", bufs=2) as spool,
            tc.tile_pool(name="qps", bufs=4, space="PSUM") as qpool,
            tc.tile_pool(name="hps", bufs=2, space="PSUM") as hpool,
            tc.tile_pool(name="rps", bufs=2, space="PSUM") as rpool,
        ):
            idx16 = cpool.tile_from(idx_d)
            dt = cpool.tile_from(dt_d)
            invev = cpool.tile_from(invev_d)
            ident = cpool.tile_from(ident_d)
            invpt = cpool.tile_from(invpt_d)
            a0inv = cpool.tile_from(a0inv_d)
            hsup = cpool.tile_from(hsup_d)
            nident = cpool.tile_from(nident_d)
            biases = cpool.tile_from(bias_d)
            stats = cpool.tile([P, NCH], f32, tag="stats")

            for sc in range(NCH):
                s0 = sc * CH

                g = gpool.tile([P, 16, CH], bf16, tag="g")
                nc.gpsimd.dma_gather(
                    g,
                    maux,
                    idx16[:, sc * (CH // 16) : (sc + 1) * (CH // 16)],
                    CH,
                    CH,
                    2048,
                    transpose=True,
                )

                # ---- q' = K^(1/1.852)*(D @ inv_even + net*PM[:, idx]) ----
                # 4 waves of 2 pipe-chunks: 2 waves in flight in PSUM so PE
                # matmuls of wave w+1 overlap the ACT/DVE chain of wave w.
                hl = wpool.tile([P, 8 * CH], bf16, tag="hl", bufs=2)
                for w in range(4):
                    absq = wpool.tile([P, 2 * CH], f32, tag="absq", bufs=3)
                    lne = wpool.tile([P, 2 * CH], f32, tag="lne", bufs=2)
                    e_t = wpool.tile([P, 2 * CH], bf16, tag="e_t", bufs=3)
                    qps = []
                    for i, pc in enumerate(range(2 * w, 2 * w + 2)):
                        qp = qpool.tile([P, CH], f32, tag="qp")
                        nc.tensor.matmul(
                            qp,
                            invev[:, (0 * 8 + pc) * P : (0 * 8 + pc + 1) * P],
                            dt[:, 0 * SC + s0 : 0 * SC + s0 + CH],
                            start=True,
                            stop=False,
                        )
                        nc.tensor.matmul(
                            qp,
                            invev[:, (1 * 8 + pc) * P : (1 * 8 + pc + 1) * P],
                            dt[:, 1 * SC + s0 : 1 * SC + s0 + CH],
                            start=False,
                            stop=False,
                        )
                        nc.tensor.matmul(
                            qp, ident, g[:, 4 + pc, :], start=False, stop=True
                        )
                        # |q| on DVE: clear the sign bit on an int32 view
                        nc.vector.tensor_scalar(
                            absq[:, i * CH : (i + 1) * CH].bitcast(mybir.dt.int32),
                            qp.bitcast(mybir.dt.int32),
                            0x7FFFFFFF,
                            None,
                            OP.bitwise_and,
                        )
                        qps.append(qp)
                    nc.scalar.activation(lne, absq, AF.Ln, bias=biases[:, 0:1])
                    nc.scalar.activation(e_t, lne, AF.Exp, scale=0.852)
                    for i, pc in enumerate(range(2 * w, 2 * w + 2)):
                        # hL = q'|q'|^0.852 — last reader of the q PSUM bank
                        nc.vector.tensor_tensor(
                            hl[:, pc * CH : (pc + 1) * CH],
                            qps[i],
                            e_t[:, i * CH : (i + 1) * CH],
                            OP.mult,
                        )

                # ---- H = hsup - hL @ inv'^T ; sq = c0*sqrt(relu(H)) ----
                rl = wpool.tile([P, 4 * CH], bf16, tag="rl", bufs=2)
                lnh = wpool.tile([P, 4 * CH], f32, tag="lnh")
                sq = wpool.tile([P, 4 * CH], bf16, tag="sq", bufs=2)
                for n_ in range(4):
                    hp = hpool.tile([P, CH], f32, tag="hp")
                    for kc in range(8):
                        nc.tensor.matmul(
                            hp,
                            invpt[:, (kc * 4 + n_) * P : (kc * 4 + n_ + 1) * P],
                            hl[:, kc * CH : (kc + 1) * CH],
                            start=(kc == 0),
                            stop=(kc == 7),
                        )
                    nc.scalar.activation(
                        rl[:, n_ * CH : (n_ + 1) * CH],
                        hp,
                        AF.Relu,
                        bias=hsup[:, n_ : n_ + 1],
                        scale=-1.0,
                    )
                nc.scalar.activation(lnh, rl, AF.Ln, bias=biases[:, 0:1])
                nc.scalar.activation(sq, lnh, AF.Exp, scale=0.5, bias=biases[:, 1:2])

                # ---- residual chunks + sum of squares ----
                # accumulate R PSUM early (PE filler during ACT/DVE chains);
                # demand folded in via -I matmul
                r_all = wpool.tile([P, 4 * CH], f32, tag="r_all", bufs=2)
                rps = []
                for n_ in range(4):
                    rp = rpool.tile([P, CH], f32, tag="rp")
                    nc.tensor.matmul(
                        rp,
                        a0inv[:, (0 * 4 + n_) * P : (0 * 4 + n_ + 1) * P],
                        dt[:, 0 * SC + s0 : 0 * SC + s0 + CH],
                        start=True,
                        stop=False,
                    )
                    nc.tensor.matmul(
                        rp,
                        a0inv[:, (1 * 4 + n_) * P : (1 * 4 + n_ + 1) * P],
                        dt[:, 1 * SC + s0 : 1 * SC + s0 + CH],
                        start=False,
                        stop=False,
                    )
                    if n_ < 2:
                        nc.tensor.matmul(
                            rp,
                            nident,
                            dt[:, n_ * SC + s0 : n_ * SC + s0 + CH],
                            start=False,
                            stop=False,
                        )
                    nc.tensor.matmul(rp, ident, g[:, 12 + n_, :], start=False, stop=True)
                    rps.append(rp)
                for n_ in range(4):
                    nsl = slice(n_ * CH, (n_ + 1) * CH)
                    dl = spool.tile([P, CH], bf16, tag="dl")
                    nc.vector.tensor_tensor(dl, g[:, n_, :], sq[:, nsl], OP.mult)
                    nc.vector.tensor_tensor(r_all[:, nsl], rps[n_], dl, OP.subtract)
                scr = wpool.tile([P, 4 * CH], bf16, tag="scr", bufs=2)
                nc.scalar.activation(
                    scr, r_all, AF.Square, accum_out=stats[:, sc : sc + 1]
                )

            nc.sync.dma_start(out_d, stats)

    nc.compile()
    return nc


def _host_prep(inputs):
    D = np.ascontiguousarray(np.asarray(inputs["D"], np.float32))
    leak = np.asarray(inputs["leak_id"]).reshape(-1).astype(np.int64)
    A0 = np.asarray(inputs["A0"], np.float32)
    inv = np.asarray(inputs["inv"], np.float32)
    M = np.asarray(inputs["M"], np.float32)
    supply = np.asarray(inputs["supply"], np.float32)
    L = np.asarray(inputs["L"], np.float32)
    d = np.asarray(inputs["d"], np.float32)
    C = np.asarray(inputs["C"], np.float32)
    a = float(np.asarray(inputs["a"]))
    Cd = float(np.asarray(inputs["Cd"]))
    W1 = np.asarray(inputs["W1"], np.float32)
    b1 = np.asarray(inputs["b1"], np.float32)
    W2 = np.asarray(inputs["W2"], np.float32)
    b2 = np.asarray(inputs["b2"], np.float32)
    W3 = np.asarray(inputs["W3"], np.float32)
    b3 = np.asarray(inputs["b3"], np.float32)
    base = np.asarray(inputs["base"], np.float32)

    # per-pipe net table (memoized MLP over the 1024 possible leak ids)
    ids = np.arange(N_PIPES, dtype=np.float32)[:, None]
    h = np.tanh(ids @ W1 + b1)
    h = np.tanh(h @ W2 + b2)
    table = base + (h @ W3 + b3)[:, 0]

    perm = np.concatenate([np.arange(0, N_NODES, 2), np.arange(1, N_NODES, 2)])
    Mp = M[perm]
    invp = inv[perm]
    inv_ev = invp[:N_DEM]  # rows of inv at even node indices

    K = 10.667 * C**-1.852 * d**-4.871 * L
    k1 = K ** (1.0 / 1.852)  # fold into q so hL = q'|q'|^0.852

    PM = inv.T @ M                        # [1024p, 1024t]
    PMn = (PM * table[None, :]) * k1[:, None]
    A0p = A0[perm]
    AMn = (A0p @ PM) * table[None, :]     # [512n, 1024t]
    A0inv = A0p @ inv_ev.T                # [512n, 256j]

    maux = np.concatenate([Mp.T, PMn.T, AMn.T], axis=1).astype(BF16)  # [1024, 2048]

    def blocks(mat, kb, mb):
        # [kb*128, mb*128] -> [128, kb*mb*128], block b = kc*mb + mc
        out = np.empty((P, kb * mb * P), np.float32)
        for kc in range(kb):
            for mc in range(mb):
                b = kc * mb + mc
                out[:, b * P : (b + 1) * P] = mat[
                    kc * P : (kc + 1) * P, mc * P : (mc + 1) * P
                ]
        return out

    invev_l = blocks(inv_ev * k1[None, :], 2, 8).astype(BF16)
    invpt_l = blocks(invp.T, 8, 4).astype(BF16)
    a0inv_l = blocks(A0inv.T, 2, 4).astype(BF16)

    hsup_l = np.ascontiguousarray((invp @ supply).reshape(4, P).T).astype(np.float32)
    ident = np.eye(P, dtype=np.float32).astype(BF16)
    nident = (-np.eye(P, dtype=np.float32)).astype(BF16)
    c0 = Cd * a * math.sqrt(2.0 * G_ACC)

    dts = []
    idxs = []
    for c in range(N_CORES):
        Dc = D[c * SC : (c + 1) * SC]  # [2048, 256]
        DT = np.ascontiguousarray(Dc.T).astype(BF16)  # [256, 2048]
        dts.append(np.concatenate([DT[:P], DT[P:]], axis=1))  # [128, 4096]
        w16 = np.zeros((16, SC // 16), np.int16)
        lc = leak[c * SC : (c + 1) * SC]
        for sc in range(NCH):
            w16[:, sc * (CH // 16) : (sc + 1) * (CH // 16)] = (
                lc[sc * CH : (sc + 1) * CH].reshape(CH // 16, 16).T
            )
        # the gather firmware's Q7 cores read the index block from their own
        # 16-partition group — replicate it across all 8 groups
        idxs.append(np.tile(w16, (8, 1)))

    shared = {
        "maux": maux,
        "invev": invev_l,
        "invpt": invpt_l,
        "a0inv": a0inv_l,
        "hsup": hsup_l,
        "ident": ident,
        "nident": nident,
    }
    return shared, dts, idxs, c0


LAST_RESULTS = None


def kernel(**inputs) -> np.ndarray:
    global LAST_RESULTS
    from concourse.bass_utils import run_bass_kernel_spmd

    shared, dts, idxs, c0 = _host_prep(inputs)

    if "nc" not in _MODULE_CACHE:
        _MODULE_CACHE["nc"] = _build_module()
    nc = _MODULE_CACHE["nc"]
    bias_arr = np.zeros((P, 2), np.float32)
    bias_arr[:, 0] = 1e-35
    bias_arr[:, 1] = math.log(c0)

    in_maps = []
    for c in range(N_CORES):
        m = dict(shared)
        m["biases"] = bias_arr
        m["dt"] = dts[c]
        m["idx16"] = idxs[c]
        in_maps.append(m)

    import os

    res = run_bass_kernel_spmd(
        nc,
        in_maps,
        core_ids=list(range(N_CORES)),
        trace=bool(os.environ.get("BASS_TRACE")),
    )
    LAST_RESULTS = res

    total = 0.0
    for r in res.results:
        total += float(r["out_stats"].astype(np.float64).sum())
    return np.float32(total / (S_TOTAL * N_NODES))


# revision 13
# speedup vs baseline: 1.6432x; 1.0201x over previous
"""Trainium2 Bass kernel for the water-network leak MSE model.

Math (reference):
    net(s)   = base[idx_s] + MLP(idx_s)                    (idx_s in [0,1024))
    y        = net*onehot(idx) @ M^T + demand              demand[:, 2j] = D[:, j]
    q        = y @ inv
    hL       = sign(q) * K * |q|^1.852,  K = 10.667 C^-1.852 d^-4.871 L
    H        = (supply - hL) @ inv^T
    d_leak   = Cd*a*sqrt(2g) * (onehot @ M^T) * sqrt(relu(H))
    out      = mean((q @ A0^T - demand - d_leak)^2)

Device strategy (8 cores, data-parallel over samples, 2048 samples/core):
  All sample-independent weight transforms are folded on the host:
    PM  = inv^T M   (so q = net * PM[:, idx] + D @ inv_even),
    AM  = A0' PM    (so q @ A0'^T = net * AM[:, idx] + D @ (A0' inv_even^T)^T),
  with the per-pipe net table pre-multiplied into PM/AM columns, and the
  Hazen-Williams coefficient folded into q itself (q' = K^{1/1.852} q, so
  hL = q'|q'|^0.852 needs no per-pipe scaling on device). Node rows are
  permuted even-first so the demand subtraction is a contiguous slice.
  On device, per 512-sample chunk (features on partitions, samples on free):
    - one transposed dma_gather pulls M^T/PM^T/AM^T columns for the chunk's
      leak ids directly into [feature, sample] layout (bf16),
    - PE: D-part matmuls (K=256) for q and the residual, identity-matmul
      injects of the gathered parts into PSUM, and the full H matmul (K=1024),
    - ACT: ln/exp power chains (natural_log_exp table set only, loaded once),
    - DVE: |q| (sign-bit clear), hL = q*e from PSUM, residual assembly,
      fused square+reduce partials,
    - Pool: gathers and d_leak elementwise.
  q is processed in two 4-bank PSUM waves so hL reads PSUM directly and the
  banks recycle (PSUM budget: 4 q + 2 H + 2 R = 8 banks).
  Each core returns [128, 16] partial sums of squares; host reduces.
"""

import math

import numpy as np
import ml_dtypes

P = 128
N_CORES = 8
S_TOTAL = 16384
SC = S_TOTAL // N_CORES  # samples per core
CH = 512                 # samples per chunk
NCH = SC // CH           # chunks per core
N_NODES = 512
N_PIPES = 1024
N_DEM = 256
G_ACC = 9.80665

BF16 = ml_dtypes.bfloat16

_MODULE_CACHE: dict = {}


def _build_module():
    import concourse.bacc as bacc
    import concourse.mybir as mybir
    import concourse.tile as tile

    f32 = mybir.dt.float32
    bf16 = mybir.dt.bfloat16
    i16 = mybir.dt.int16
    AF = mybir.ActivationFunctionType
    OP = mybir.AluOpType

    nc = bacc.Bacc(trn_type="TRN2", target_bir_lowering=False, debug=False)

    # All our activations (Abs/Relu/Square/Ln/Exp) live in the
    # natural_log_exp_and_others table set, but the table-load pass maps each
    # func to the first set containing it, ping-ponging between exp_and_others
    # and natural_log (25 table loads, ~40us of ACT). Strip our funcs from
    # every other set so the pass converges on the one shared set.
    import types as _types
    from concourse.hw_specs import get_activation_tables as _gat
    import bass_rust as _bass_rust

    _OURS = {AF.Abs, AF.Relu, AF.Square, AF.Ln, AF.Exp, AF.Identity, AF.Copy,
             AF.Sign, AF.MemsetZero}

    def _patched_act_table_loads(self):
        has_activation = any(
            isinstance(i, mybir.InstActivation)
            for b in self.main_func.blocks
            for i in b.instructions
        )
        if not has_activation:
            return
        tables = []
        for name, fns in _gat(self.m.arch).items():
            if name != "natural_log_exp_and_others":
                fns = fns - _OURS
            tables.append((name, fns))
        _bass_rust.insert_act_table_loads(self, tables)

    nc.insert_act_table_loads = _types.MethodType(_patched_act_table_loads, nc)

    maux = nc.dram_tensor("maux", [N_PIPES, 2048], bf16, kind="ExternalInput").ap()
    invev_d = nc.dram_tensor("invev", [P, 16 * P], bf16, kind="ExternalInput").ap()
    invpt_d = nc.dram_tensor("invpt", [P, 32 * P], bf16, kind="ExternalInput").ap()
    a0inv_d = nc.dram_tensor("a0inv", [P, 8 * P], bf16, kind="ExternalInput").ap()
    dt_d = nc.dram_tensor("dt", [P, 2 * SC], bf16, kind="ExternalInput").ap()
    hsup_d = nc.dram_tensor("hsup", [P, 4], f32, kind="ExternalInput").ap()
    ident_d = nc.dram_tensor("ident", [P, P], bf16, kind="ExternalInput").ap()
    nident_d = nc.dram_tensor("nident", [P, P], bf16, kind="ExternalInput").ap()
    idx_d = nc.dram_tensor("idx16", [P, SC // 16], i16, kind="ExternalInput").ap()
    bias_d = nc.dram_tensor("biases", [P, 2], f32, kind="ExternalInput").ap()
    out_d = nc.dram_tensor("out_stats", [P, NCH], f32, kind="ExternalOutput").ap()

    with tile.TileContext(nc) as tc:
        with (
            tc.tile_pool(name="const", bufs=1) as cpool,
            tc.tile_pool(name="gat", bufs=3) as gpool,
            tc.tile_pool(name="work", bufs=1) as wpool,
            tc.tile_pool(name="small", bufs=2) as spool,
            tc.tile_pool(name="qps", bufs=3, space="PSUM") as qpool,
            tc.tile_pool(name="hps", bufs=3, space="PSUM") as hpool,
            tc.tile_pool(name="rps", bufs=2, space="PSUM") as rpool,
        ):
            idx16 = cpool.tile_from(idx_d, forced_dma_engine=mybir.EngineType.Pool)
            dt = cpool.tile_from(dt_d)
            invev = cpool.tile_from(invev_d)
            ident = cpool.tile_from(ident_d)
            invpt = cpool.tile_from(invpt_d)
            a0inv = cpool.tile_from(a0inv_d)
            hsup = cpool.tile_from(hsup_d)
            nident = cpool.tile_from(nident_d)
            biases = cpool.tile_from(bias_d)
            stats = cpool.tile([P, NCH], f32, tag="stats")

            for sc in range(NCH):
                s0 = sc * CH

                g = gpool.tile([P, 16, CH], bf16, tag="g")
                nc.gpsimd.dma_gather(
                    g,
                    maux,
                    idx16[:, sc * (CH // 16) : (sc + 1) * (CH // 16)],
                    CH,
                    CH,
                    2048,
                    transpose=True,
                )

                # ---- q' = K^(1/1.852)*(D @ inv_even + net*PM[:, idx]) ----
                # D-part matmuls into PSUM; DVE adds the gathered net*PM part
                # while draining to SBUF bf16 (releases the bank); then one
                # big |.| / ln / exp / hL chain over all 8 pipe chunks.
                qsb = wpool.tile([P, 8 * CH], bf16, tag="qsb", bufs=2)
                absq = wpool.tile([P, 8 * CH], bf16, tag="absq", bufs=2)
                lne = wpool.tile([P, 8 * CH], f32, tag="lne")
                e_t = wpool.tile([P, 8 * CH], bf16, tag="e_t", bufs=2)
                hl = wpool.tile([P, 8 * CH], bf16, tag="hl", bufs=2)
                for pc in range(8):
                    qp = qpool.tile([P, CH], f32, tag="qp")
                    nc.tensor.matmul(
                        qp,
                        invev[:, (0 * 8 + pc) * P : (0 * 8 + pc + 1) * P],
                        dt[:, 0 * SC + s0 : 0 * SC + s0 + CH],
                        start=True,
                        stop=False,
                    )
                    nc.tensor.matmul(
                        qp,
                        invev[:, (1 * 8 + pc) * P : (1 * 8 + pc + 1) * P],
                        dt[:, 1 * SC + s0 : 1 * SC + s0 + CH],
                        start=False,
                        stop=True,
                    )
                    # q = Dq + net*PM[:, idx]; drains + releases the PSUM bank
                    nc.vector.tensor_tensor(
                        qsb[:, pc * CH : (pc + 1) * CH], qp, g[:, 4 + pc, :], OP.add
                    )
                nc.vector.tensor_scalar(
                    absq.bitcast(mybir.dt.int16),
                    qsb.bitcast(mybir.dt.int16),
                    0x7FFF,
                    None,
                    OP.bitwise_and,
                )
                nc.scalar.activation(lne, absq, AF.Ln, bias=biases[:, 0:1])
                nc.scalar.activation(e_t, lne, AF.Exp, scale=0.852)
                # hL = q'|q'|^0.852
                nc.vector.tensor_tensor(hl, qsb, e_t, OP.mult)

                # ---- H = hsup - hL @ inv'^T ; sq = c0*sqrt(relu(H)) ----
                rl = wpool.tile([P, 4 * CH], bf16, tag="rl", bufs=2)
                lnh = wpool.tile([P, 4 * CH], f32, tag="lnh")
                sq = wpool.tile([P, 4 * CH], bf16, tag="sq", bufs=2)
                for n_ in range(4):
                    hp = hpool.tile([P, CH], f32, tag="hp")
                    for kc in range(8):
                        nc.tensor.matmul(
                            hp,
                            invpt[:, (kc * 4 + n_) * P : (kc * 4 + n_ + 1) * P],
                            hl[:, kc * CH : (kc + 1) * CH],
                            start=(kc == 0),
                            stop=(kc == 7),
                        )
                    nc.scalar.activation(
                        rl[:, n_ * CH : (n_ + 1) * CH],
                        hp,
                        AF.Relu,
                        bias=hsup[:, n_ : n_ + 1],
                        scale=-1.0,
                    )
                nc.scalar.activation(lnh, rl, AF.Ln, bias=biases[:, 0:1])
                nc.scalar.activation(sq, lnh, AF.Exp, scale=0.5, bias=biases[:, 1:2])

                # ---- residual chunks + sum of squares ----
                # rp = D-part (+ -I demand fold); DVE adds gathered net*AM and
                # subtracts d_leak during the drain
                r_all = wpool.tile([P, 4 * CH], f32, tag="r_all", bufs=2)
                rps = []
                for n_ in range(4):
                    rp = rpool.tile([P, CH], f32, tag="rp")
                    nc.tensor.matmul(
                        rp,
                        a0inv[:, (0 * 4 + n_) * P : (0 * 4 + n_ + 1) * P],
                        dt[:, 0 * SC + s0 : 0 * SC + s0 + CH],
                        start=True,
                        stop=False,
                    )
                    nc.tensor.matmul(
                        rp,
                        a0inv[:, (1 * 4 + n_) * P : (1 * 4 + n_ + 1) * P],
                        dt[:, 1 * SC + s0 : 1 * SC + s0 + CH],
                        start=False,
                        stop=(n_ >= 2),
                    )
                    if n_ < 2:
                        nc.tensor.matmul(
                            rp,
                            nident,
                            dt[:, n_ * SC + s0 : n_ * SC + s0 + CH],
                            start=False,
                            stop=True,
                        )
                    rps.append(rp)
                for n_ in range(4):
                    nsl = slice(n_ * CH, (n_ + 1) * CH)
                    dl = spool.tile([P, CH], bf16, tag="dl")
                    nc.vector.tensor_tensor(dl, g[:, n_, :], sq[:, nsl], OP.mult)
                    amdl = spool.tile([P, CH], bf16, tag="amdl")
                    nc.vector.tensor_tensor(amdl, g[:, 12 + n_, :], dl, OP.subtract)
                    nc.vector.tensor_tensor(r_all[:, nsl], rps[n_], amdl, OP.add)
                scr = wpool.tile([P, 4 * CH], bf16, tag="scr", bufs=2)
                nc.scalar.activation(
                    scr, r_all, AF.Square, accum_out=stats[:, sc : sc + 1]
                )
            nc.sync.dma_start(out_d, stats)

    nc.compile()
    return nc


def _host_prep(inputs):
    D = np.ascontiguousarray(np.asarray(inputs["D"], np.float32))
    leak = np.asarray(inputs["leak_id"]).reshape(-1).astype(np.int64)
    A0 = np.asarray(inputs["A0"], np.float32)
    inv = np.asarray(inputs["inv"], np.float32)
    M = np.asarray(inputs["M"], np.float32)
    supply = np.asarray(inputs["supply"], np.float32)
    L = np.asarray(inputs["L"], np.float32)
    d = np.asarray(inputs["d"], np.float32)
    C = np.asarray(inputs["C"], np.float32)
    a = float(np.asarray(inputs["a"]))
    Cd = float(np.asarray(inputs["Cd"]))
    W1 = np.asarray(inputs["W1"], np.float32)
    b1 = np.asarray(inputs["b1"], np.float32)
    W2 = np.asarray(inputs["W2"], np.float32)
    b2 = np.asarray(inputs["b2"], np.float32)
    W3 = np.asarray(inputs["W3"], np.float32)
    b3 = np.asarray(inputs["b3"], np.float32)
    base = np.asarray(inputs["base"], np.float32)

    # per-pipe net table (memoized MLP over the 1024 possible leak ids)
    ids = np.arange(N_PIPES, dtype=np.float32)[:, None]
    h = np.tanh(ids @ W1 + b1)
    h = np.tanh(h @ W2 + b2)
    table = base + (h @ W3 + b3)[:, 0]

    perm = np.concatenate([np.arange(0, N_NODES, 2), np.arange(1, N_NODES, 2)])
    Mp = M[perm]
    invp = inv[perm]
    inv_ev = invp[:N_DEM]  # rows of inv at even node indices

    K = 10.667 * C**-1.852 * d**-4.871 * L
    k1 = K ** (1.0 / 1.852)  # fold into q so hL = q'|q'|^0.852

    PM = inv.T @ M                        # [1024p, 1024t]
    PMn = (PM * table[None, :]) * k1[:, None]
    A0p = A0[perm]
    AMn = (A0p @ PM) * table[None, :]     # [512n, 1024t]
    A0inv = A0p @ inv_ev.T                # [512n, 256j]

    maux = np.concatenate([Mp.T, PMn.T, AMn.T], axis=1).astype(BF16)  # [1024, 2048]

    def blocks(mat, kb, mb):
        # [kb*128, mb*128] -> [128, kb*mb*128], block b = kc*mb + mc
        out = np.empty((P, kb * mb * P), np.float32)
        for kc in range(kb):
            for mc in range(mb):
                b = kc * mb + mc
                out[:, b * P : (b + 1) * P] = mat[
                    kc * P : (kc + 1) * P, mc * P : (mc + 1) * P
                ]
        return out

    invev_l = blocks(inv_ev * k1[None, :], 2, 8).astype(BF16)
    invpt_l = blocks(invp.T, 8, 4).astype(BF16)
    a0inv_l = blocks(A0inv.T, 2, 4).astype(BF16)

    hsup_l = np.ascontiguousarray((invp @ supply).reshape(4, P).T).astype(np.float32)
    ident = np.eye(P, dtype=np.float32).astype(BF16)
    nident = (-np.eye(P, dtype=np.float32)).astype(BF16)
    c0 = Cd * a * math.sqrt(2.0 * G_ACC)

    dts = []
    idxs = []
    for c in range(N_CORES):
        Dc = D[c * SC : (c + 1) * SC]  # [2048, 256]
        DT = np.ascontiguousarray(Dc.T).astype(BF16)  # [256, 2048]
        dts.append(np.concatenate([DT[:P], DT[P:]], axis=1))  # [128, 4096]
        w16 = np.zeros((16, SC // 16), np.int16)
        lc = leak[c * SC : (c + 1) * SC]
        for sc in range(NCH):
            w16[:, sc * (CH // 16) : (sc + 1) * (CH // 16)] = (
                lc[sc * CH : (sc + 1) * CH].reshape(CH // 16, 16).T
            )
        # the gather firmware's Q7 cores read the index block from their own
        # 16-partition group — replicate it across all 8 groups
        idxs.append(np.tile(w16, (8, 1)))

    shared = {
        "maux": maux,
        "invev": invev_l,
        "invpt": invpt_l,
        "a0inv": a0inv_l,
        "hsup": hsup_l,
        "ident": ident,
        "nident": nident,
    }
    return shared, dts, idxs, c0


LAST_RESULTS = None


def kernel(**inputs) -> np.ndarray:
    global LAST_RESULTS
    from concourse.bass_utils import run_bass_kernel_spmd

    shared, dts, idxs, c0 = _host_prep(inputs)

    if "nc" not in _MODULE_CACHE:
        _MODULE_CACHE["nc"] = _build_module()
    nc = _MODULE_CACHE["nc"]
    bias_arr = np.zeros((P, 2), np.float32)
    bias_arr[:, 0] = 1e-35
    bias_arr[:, 1] = math.log(c0)

    in_maps = []
    for c in range(N_CORES):
        m = dict(shared)
        m["biases"] = bias_arr
        m["dt"] = dts[c]
        m["idx16"] = idxs[c]
        in_maps.append(m)

    import os

    res = run_bass_kernel_spmd(
        nc,
        in_maps,
        core_ids=list(range(N_CORES)),
        trace=bool(os.environ.get("BASS_TRACE")),
    )
    LAST_RESULTS = res

    total = 0.0
    for r in res.results:
        total += float(r["out_stats"].astype(np.float64).sum())
    return np.float32(total / (S_TOTAL * N_NODES))
